# Initial kernel scaffold
#
"""Trainium2 Bass kernel for nn_AttentionLayer (B=4, S=2048, D=1024, fp32).

Sharding: 8 cores = 4 batches x 2 query-halves. Each core computes the
attention output for 1024 query rows of one batch, with no collectives.

Per-core math (all on-device, fp32r matmuls, fp32 softmax):
  A   = W_q @ W_k^T                     [D, D]
  T^T = A^T @ x_q^T                     [D, SQ]   (T = x_q @ A)
  S   = T @ x_kv^T                      [SQ, SKV] == q @ k^T exactly
  P   = exp(S - rowmax)                 (rowsum kept for final scale)
  U^T = x_kv^T @ P^T                    [D, SQ]   (U = P @ x_kv)
  O   = (U @ W_v) * (1/rowsum)          [SQ, D]  == softmax(S) @ v

The identity (x W_q)(x W_k)^T == x (W_q W_k^T) x^T and
P (x W_v) == (P x) W_v removes all duplicated projection work across
cores: 15.05 GFLOP/core == total/8.
"""

import numpy as np

import concourse.bass as bass
import concourse.mybir as mybir
import concourse.tile as tile
from concourse import bacc
from concourse.bass_utils import run_bass_kernel_spmd
from concourse.masks import make_identity
from contextlib import ExitStack

F32 = mybir.dt.float32
F32R = mybir.dt.float32r
AX = mybir.AxisListType
ACT = mybir.ActivationFunctionType

B, S, D = 4, 2048, 1024
SQ = 1024           # query rows per core
SKV = 2048          # kv rows per core (full batch)
DT = D // 128       # 8 d/e tiles
QT = SQ // 128      # 8 q tiles
KVT = SKV // 128    # 16 kv tiles
NCH = 512           # matmul free-dim chunk
NQC = SQ // NCH     # 2 q chunks
NKC = SKV // NCH    # 4 kv chunks
NDC = D // NCH      # 2 d chunks


def build_nc():
    nc = bacc.Bacc("TRN2", target_bir_lowering=False, debug=False, num_devices=8)

    # DRAM inputs (host pre-layouts; all fp32 bits, declared f32r for the PE)
    wqTS_d = nc.dram_tensor("wqTS", [DT, D, 128], F32R, kind="ExternalInput")
    wkT_d = nc.dram_tensor("wkT", [D, D], F32R, kind="ExternalInput")
    wv_d = nc.dram_tensor("wv", [D, D], F32R, kind="ExternalInput")
    xqT_d = nc.dram_tensor("xqT", [D, SQ], F32R, kind="ExternalInput")
    xkvT_d = nc.dram_tensor("xkvT", [D, SKV], F32R, kind="ExternalInput")
    xkvS_d = nc.dram_tensor("xkvS", [DT, SKV, 128], F32R, kind="ExternalInput")
    out_d = nc.dram_tensor("out", [SQ, D], F32, kind="ExternalOutput")

    with tile.TileContext(nc) as tc, ExitStack() as es:
        # --- PSUM pools: 4 + 2 + 2 banks
        ps_acc = es.enter_context(tc.tile_pool(name="ps_acc", bufs=4, space="PSUM"))
        ps_tr = es.enter_context(tc.tile_pool(name="ps_tr", bufs=2, space="PSUM"))
        ps_uo = es.enter_context(tc.tile_pool(name="ps_uo", bufs=2, space="PSUM"))

        # --- persistent SBUF: T^T, identity, softmax stats, recip
        pers = es.enter_context(tc.tile_pool(name="pers", bufs=1))
        stat = es.enter_context(tc.tile_pool(name="stat", bufs=3))
        TT_sb = pers.tile([128, DT * SQ], F32R, tag="TT")
        ident = pers.tile([128, 128], F32, tag="ident")
        make_identity(nc, ident[:])
        recip_sb = pers.tile([128, QT], F32, tag="recip")

        # ============ phase 1: A = Wq @ Wk^T ============
        es_A = ExitStack()
        pA = es_A.enter_context(tc.tile_pool(name="pA", bufs=1))
        pXQ = es_A.enter_context(tc.tile_pool(name="pXQ", bufs=1))
        A_sb = pA.tile([128, DT * D], F32R, tag="A")
        xqT_sb = pXQ.tile([128, DT * SQ], F32R, tag="xqT")
        for dt in range(DT):
            nc.sync.dma_start(
                xqT_sb[:, dt * SQ:(dt + 1) * SQ], xqT_d.ap()[dt * 128:(dt + 1) * 128, :]
            )

        with tc.tile_pool(name="p1", bufs=1) as p1, \
             tc.tile_pool(name="p1s", bufs=2) as p1s:
            wkT_sb = p1.tile([128, DT * D], F32R, tag="wkT")
            for ct in range(DT):
                nc.sync.dma_start(
                    wkT_sb[:, ct * D:(ct + 1) * D], wkT_d.ap()[ct * 128:(ct + 1) * 128, :]
                )
            for at in range(DT):
                wq_strip = p1s.tile([128, DT, 128], F32R, tag="wqs")
                nc.sync.dma_start(
                    wq_strip[:],
                    wqTS_d.ap()[at].rearrange("(ct p) a -> p ct a", p=128),
                )
                for bc in range(NDC):
                    a_ps = ps_acc.tile([128, NCH], F32, tag="acc")
                    for ct in range(DT):
                        nc.tensor.matmul(
                            a_ps[:],
                            wq_strip[:, ct, :],
                            wkT_sb[:, ct * D + bc * NCH: ct * D + (bc + 1) * NCH],
                            start=(ct == 0),
                            stop=(ct == DT - 1),
                        )
                    nc.vector.tensor_copy(
                        A_sb[:, at * D + bc * NCH: at * D + (bc + 1) * NCH], a_ps[:]
                    )

        # ============ phase 2: T^T = A^T @ xq^T ============
        es_kvT = ExitStack()
        pKVT = es_kvT.enter_context(tc.tile_pool(name="pKVT", bufs=1))
        xkvT_sb = pKVT.tile([128, DT * SKV], F32R, tag="xkvT")
        for et in range(DT):
            nc.sync.dma_start(
                xkvT_sb[:, et * SKV:(et + 1) * SKV],
                xkvT_d.ap()[et * 128:(et + 1) * 128, :],
            )
        for et in range(DT):
            for qc in range(NQC):
                t_ps = ps_acc.tile([128, NCH], F32, tag="acc")
                for dt in range(DT):
                    nc.tensor.matmul(
                        t_ps[:],
                        A_sb[:, dt * D + et * 128: dt * D + (et + 1) * 128],
                        xqT_sb[:, dt * SQ + qc * NCH: dt * SQ + (qc + 1) * NCH],
                        start=(dt == 0),
                        stop=(dt == DT - 1),
                    )
                nc.vector.tensor_copy(
                    TT_sb[:, et * SQ + qc * NCH: et * SQ + (qc + 1) * NCH], t_ps[:]
                )
        es_A.close()  # free A, xqT

        # ============ phase 3: attention (S -> softmax -> P^T -> U^T) ============
        pUT = es.enter_context(tc.tile_pool(name="pUT", bufs=1))
        UT_sb = pUT.tile([128, DT * SQ], F32R, tag="UT")
        with tc.tile_pool(name="p3", bufs=1) as p3, \
             tc.tile_pool(name="p3p", bufs=6) as p3p, \
             tc.tile_pool(name="p3s", bufs=2) as p3s:
            for qc in range(NQC):
                PT_sb = p3.tile([128, KVT * NCH], F32R, tag="PT")
                for qi in range(QT // NQC):
                    qt = qc * (QT // NQC) + qi
                    # S chunks into PSUM
                    s_ps = []
                    for kc in range(NKC):
                        sp = ps_acc.tile([128, NCH], F32, tag="acc")
                        for et in range(DT):
                            nc.tensor.matmul(
                                sp[:],
                                TT_sb[:, et * SQ + qt * 128: et * SQ + (qt + 1) * 128],
                                xkvT_sb[:, et * SKV + kc * NCH: et * SKV + (kc + 1) * NCH],
                                start=(et == 0),
                                stop=(et == DT - 1),
                            )
                        s_ps.append(sp)
                    # softmax stats
                    m4 = stat.tile([128, NKC], F32, tag="m4")
                    for kc in range(NKC):
                        nc.vector.reduce_max(m4[:, kc:kc + 1], s_ps[kc][:], axis=AX.X)
                    negmax = stat.tile([128, 1], F32, tag="negmax")
                    nc.vector.reduce_max(negmax[:], m4[:], axis=AX.X, negate=True)
                    rs4 = stat.tile([128, NKC], F32, tag="rs4")
                    p_ch = []
                    for kc in range(NKC):
                        pc = p3p.tile([128, NCH], F32, tag="p")
                        nc.scalar.activation(
                            pc[:], s_ps[kc][:], ACT.Exp,
                            bias=negmax[:], accum_out=rs4[:, kc:kc + 1],
                        )
                        p_ch.append(pc)
                    rs1 = stat.tile([128, 1], F32, tag="rs1")
                    nc.vector.reduce_sum(rs1[:], rs4[:], axis=AX.X)
                    nc.vector.reciprocal(recip_sb[:, qt:qt + 1], rs1[:])
                    # transpose P tiles -> PT
                    for kvt in range(KVT):
                        kc, j = divmod(kvt, NKC)
                        # P chunk kc holds kv columns [kc*NCH, (kc+1)*NCH)
                        kc, j = kvt // 4, kvt % 4
                        tp = ps_tr.tile([128, 128], F32, tag="tr")
                        nc.tensor.transpose(
                            tp[:], p_ch[kc][:, j * 128:(j + 1) * 128], ident[:]
                        )
                        nc.any.tensor_copy(
                            PT_sb[:, kvt * NCH + qi * 128: kvt * NCH + (qi + 1) * 128],
                            tp[:],
                        )
                # U^T for this q-chunk
                for et in range(DT):
                    strip = p3s.tile([128, KVT, 128], F32R, tag="xs")
                    nc.sync.dma_start(
                        strip[:],
                        xkvS_d.ap()[et].rearrange("(kvt p) c -> p kvt c", p=128),
                    )
                    u_ps = ps_uo.tile([128, NCH], F32, tag="uo")
                    for kvt in range(KVT):
                        nc.tensor.matmul(
                            u_ps[:],
                            strip[:, kvt, :],
                            PT_sb[:, kvt * NCH:(kvt + 1) * NCH],
                            start=(kvt == 0),
                            stop=(kvt == KVT - 1),
                        )
                    nc.vector.tensor_copy(
                        UT_sb[:, et * SQ + qc * NCH: et * SQ + (qc + 1) * NCH], u_ps[:]
                    )
        es_kvT.close()  # free xkvT

        # ============ phase 4: O = (U @ Wv) / rowsum ============
        with tc.tile_pool(name="p4", bufs=1) as p4, \
             tc.tile_pool(name="p4o", bufs=4) as p4o:
            wv_sb = p4.tile([128, DT * D], F32R, tag="wv")
            for et in range(DT):
                nc.sync.dma_start(
                    wv_sb[:, et * D:(et + 1) * D], wv_d.ap()[et * 128:(et + 1) * 128, :]
                )
            for qt in range(QT):
                for dc in range(NDC):
                    o_ps = ps_uo.tile([128, NCH], F32, tag="uo")
                    for et in range(DT):
                        nc.tensor.matmul(
                            o_ps[:],
                            UT_sb[:, et * SQ + qt * 128: et * SQ + (qt + 1) * 128],
                            wv_sb[:, et * D + dc * NCH: et * D + (dc + 1) * NCH],
                            start=(et == 0),
                            stop=(et == DT - 1),
                        )
                    o_sb = p4o.tile([128, NCH], F32, tag="o")
                    nc.scalar.mul(o_sb[:], o_ps[:], mul=recip_sb[:, qt:qt + 1])
                    nc.sync.dma_start(
                        out_d.ap()[qt * 128:(qt + 1) * 128, dc * NCH:(dc + 1) * NCH],
                        o_sb[:],
                    )

    nc.compile()
    return nc


_NC_CACHE = None


def get_nc():
    global _NC_CACHE
    if _NC_CACHE is None:
        _NC_CACHE = build_nc()
    return _NC_CACHE


def make_in_maps(inputs, W_query, W_key, W_value):
    x = np.ascontiguousarray(np.asarray(inputs, dtype=np.float32))
    Wq = np.asarray(W_query, dtype=np.float32)
    Wk = np.asarray(W_key, dtype=np.float32)
    Wv = np.ascontiguousarray(np.asarray(W_value, dtype=np.float32))

    # wqTS[at] = Wq^T[:, at*128:(at+1)*128]  ([D(c), 128(a)])
    wqTS = np.ascontiguousarray(Wq.reshape(DT, 128, D).transpose(0, 2, 1))
    wkT = np.ascontiguousarray(Wk.T)

    in_maps = []
    for b in range(B):
        xb = x[b]                                      # [SKV, D]
        xkvT = np.ascontiguousarray(xb.T)              # [D, SKV]
        xkvS = np.ascontiguousarray(
            xb.reshape(SKV, DT, 128).transpose(1, 0, 2)
        )                                              # [DT, SKV, 128]
        for h in range(2):
            xqT = np.ascontiguousarray(xkvT[:, h * SQ:(h + 1) * SQ])
            in_maps.append({
                "wqTS": wqTS, "wkT": wkT, "wv": Wv,
                "xqT": xqT, "xkvT": xkvT, "xkvS": xkvS,
            })
    return in_maps


def kernel(inputs, W_query, W_key, W_value):
    nc = get_nc()
    in_maps = make_in_maps(inputs, W_query, W_key, W_value)
    res = run_bass_kernel_spmd(nc, in_maps, core_ids=list(range(8)))
    out = np.empty((B, S, D), dtype=np.float32)
    for b in range(B):
        for h in range(2):
            out[b, h * SQ:(h + 1) * SQ, :] = res.results[2 * b + h]["out"]
    return out


# revision 8
# speedup vs baseline: 8.6298x; 8.6298x over previous
"""Trainium2 Bass kernel for nn_AttentionLayer (B=4, S=2048, D=1024, fp32).

Sharding: 8 cores = 4 batches x 2 query-halves. Each core computes the
attention output for 1024 query rows of one batch, with no collectives.

Per-core math (fp32r matmuls, fp32 softmax):
  A   = W_q @ W_k^T                     [D, D]
  T^T = A^T @ x_q^T                     [D, SQ]   (T = x_q @ A)
  S   = T @ x_kv^T                      [SQ, SKV] == q @ k^T exactly
  P   = exp(S - rowmax)                 (rowsum kept for final scale)
  U^T = x_kv^T @ P^T                    [D, SQ]   (U = P @ x_kv)
  O   = (U @ W_v) * (1/rowsum)          [SQ, D]  == softmax(S) @ v

The identities (x W_q)(x W_k)^T == x (W_q W_k^T) x^T and
P (x W_v) == (P x) W_v remove all duplicated projection work across
cores: 15.05 GFLOP/core == total/8.

The host rolls the kv axis per core so this core's query rows occupy
kv positions [0, SQ) — softmax and the P@x contraction are invariant
to kv order, and it lets one SPMD program serve both query-halves.
"""

import numpy as np

import concourse.bass as bass
import concourse.mybir as mybir
import concourse.tile as tile
from concourse import bacc
from concourse.bass_utils import run_bass_kernel_spmd
from concourse.masks import make_identity
from contextlib import ExitStack

F32 = mybir.dt.float32
F32R = mybir.dt.float32r
AX = mybir.AxisListType
ACT = mybir.ActivationFunctionType

B, S, D = 4, 2048, 1024
SQ = 1024           # query rows per core
SKV = 2048          # kv rows per core (full batch)
DT = D // 128       # 8 d/e tiles
QT = SQ // 128      # 8 q tiles
KVT = SKV // 128    # 16 kv tiles
NCH = 512           # matmul free-dim chunk
NQC = SQ // NCH     # 2 q chunks
NKC = SKV // NCH    # 4 kv chunks
NDC = D // NCH      # 2 d chunks


def build_nc(repeat=1):
    nc = bacc.Bacc("TRN2", target_bir_lowering=False, debug=False, num_devices=8)

    # DRAM inputs (host pre-layouts; fp32 bits, declared f32r for the PE)
    # A = W_q @ W_k^T is folded on the host (weight-only preprocessing).
    A_d = nc.dram_tensor("A", [D, D], F32R, kind="ExternalInput")
    wv_d = nc.dram_tensor("wv", [D, D], F32R, kind="ExternalInput")
    xkvT_d = nc.dram_tensor("xkvT", [D, SKV], F32R, kind="ExternalInput")
    xkvS_d = nc.dram_tensor("xkvS", [DT, SKV, 128], F32R, kind="ExternalInput")
    out_d = nc.dram_tensor("out", [SQ, D], F32, kind="ExternalOutput")

    with tile.TileContext(nc) as tc, ExitStack() as es:
        # --- PSUM pools: 5 banks for accumulation chains + 3 shared
        # (transpose outputs and U/O accumulators never need slots at the
        # same moment, so they share one 3-buf tag)
        ps_acc = es.enter_context(tc.tile_pool(name="ps_acc", bufs=5, space="PSUM"))
        ps_x = es.enter_context(tc.tile_pool(name="ps_x", bufs=3, space="PSUM"))
        ps_tr = ps_x
        ps_uo = ps_x

        # --- shared SBUF
        pers = es.enter_context(tc.tile_pool(name="pers", bufs=1))
        stat = es.enter_context(tc.tile_pool(name="stat", bufs=3))
        rp = es.enter_context(tc.tile_pool(name="rp", bufs=2))
        ident = pers.tile([128, 128], F32, tag="ident")
        make_identity(nc, ident[:])

        for _rep in range(repeat):
            _emit_rep(nc, tc, _rep, ps_acc, ps_tr, ps_uo, stat, rp, ident,
                      A_d, wv_d, xkvT_d, xkvS_d, out_d)

    nc.compile()
    return nc


def _emit_rep(nc, tc, rep, ps_acc, ps_tr, ps_uo, stat, rp, ident,
              A_d, wv_d, xkvT_d, xkvS_d, out_d):
    with ExitStack() as es:
        recip_sb = rp.tile([128, QT], F32, tag="recip")
        pTT = es.enter_context(tc.tile_pool(name=f"pTT{rep}", bufs=1))
        TT_sb = pTT.tile([128, DT * SQ], F32R, tag="TT")

        # xkv^T resident for phases 2-3; DMA streams in from t=0
        pKVT = es.enter_context(tc.tile_pool(name=f"pKVT{rep}", bufs=1))
        xkvT_sb = pKVT.tile([128, DT * SKV], F32R, tag="xkvT")

        # ============ phases 1+2: T^T = A^T @ xq^T ============
        # DMA order: A d-tiles interleaved with the phase-2-needed first
        # kv columns of xkv^T, then the rest kc-major.
        with tc.tile_pool(name=f"pA{rep}", bufs=1) as pA:
            A_sb = pA.tile([128, DT * D], F32R, tag="A")
            for dt in range(DT):
                nc.sync.dma_start(
                    A_sb[:, dt * D:(dt + 1) * D],
                    A_d.ap()[dt * 128:(dt + 1) * 128, :],
                )
                nc.sync.dma_start(
                    xkvT_sb[:, dt * SKV: dt * SKV + NCH],
                    xkvT_d.ap()[dt * 128:(dt + 1) * 128, 0:NCH],
                )
            for kc in range(1, NKC):
                for et in range(DT):
                    nc.sync.dma_start(
                        xkvT_sb[:, et * SKV + kc * NCH: et * SKV + (kc + 1) * NCH],
                        xkvT_d.ap()[et * 128:(et + 1) * 128, kc * NCH:(kc + 1) * NCH],
                    )
            # qc-outer so S over the first q-tiles can start mid-phase
            for qc in range(NQC):
                for et in range(DT):
                    t_ps = ps_acc.tile([128, NCH], F32, tag="acc")
                    for dt in range(DT):
                        nc.tensor.matmul(
                            t_ps[:],
                            A_sb[:, dt * D + et * 128: dt * D + (et + 1) * 128],
                            xkvT_sb[:, dt * SKV + qc * NCH: dt * SKV + (qc + 1) * NCH],
                            start=(dt == 0),
                            stop=(dt == DT - 1),
                        )
                    nc.vector.tensor_copy(
                        TT_sb[:, et * SQ + qc * NCH: et * SQ + (qc + 1) * NCH],
                        t_ps[:],
                    )

        # ============ phase 3: attention (S -> softmax -> P^T -> U^T) ============
        pUT = es.enter_context(tc.tile_pool(name=f"pUT{rep}", bufs=1))
        UT_sb = pUT.tile([128, DT * SQ], F32R, tag="UT")
        with tc.tile_pool(name=f"p3{rep}", bufs=1) as p3, \
             tc.tile_pool(name=f"p3p{rep}", bufs=6) as p3p, \
             tc.tile_pool(name=f"p3s{rep}", bufs=2) as p3s:
            for qc in range(NQC):
                PT_sb = p3.tile([128, KVT * NCH], F32R, tag="PT")
                for qi in range(QT // NQC):
                    qt = qc * (QT // NQC) + qi
                    # S chunks into PSUM
                    s_ps = []
                    for kc in range(NKC):
                        sp = ps_acc.tile([128, NCH], F32, tag="acc")
                        for et in range(DT):
                            nc.tensor.matmul(
                                sp[:],
                                TT_sb[:, et * SQ + qt * 128: et * SQ + (qt + 1) * 128],
                                xkvT_sb[:, et * SKV + kc * NCH: et * SKV + (kc + 1) * NCH],
                                start=(et == 0),
                                stop=(et == DT - 1),
                            )
                        s_ps.append(sp)
                    # softmax stats
                    m4 = stat.tile([128, NKC], F32, tag="m4")
                    for kc in range(NKC):
                        nc.vector.reduce_max(m4[:, kc:kc + 1], s_ps[kc][:], axis=AX.X)
                    negmax = stat.tile([128, 1], F32, tag="negmax")
                    nc.vector.reduce_max(negmax[:], m4[:], axis=AX.X, negate=True)
                    rs4 = stat.tile([128, NKC], F32, tag="rs4")
                    p_ch = []
                    for kc in range(NKC):
                        pc = p3p.tile([128, NCH], F32, tag="p")
                        nc.scalar.activation(
                            pc[:], s_ps[kc][:], ACT.Exp,
                            bias=negmax[:], accum_out=rs4[:, kc:kc + 1],
                        )
                        p_ch.append(pc)
                    rs1 = stat.tile([128, 1], F32, tag="rs1")
                    nc.vector.reduce_sum(rs1[:], rs4[:], axis=AX.X)
                    nc.vector.reciprocal(recip_sb[:, qt:qt + 1], rs1[:])
                    # transpose P tiles -> PT
                    for kvt in range(KVT):
                        kc, j = divmod(kvt, NKC)
                        tp = ps_tr.tile([128, 128], F32, tag="x")
                        nc.tensor.transpose(
                            tp[:], p_ch[kc][:, j * 128:(j + 1) * 128], ident[:]
                        )
                        nc.any.tensor_copy(
                            PT_sb[:, kvt * NCH + qi * 128: kvt * NCH + (qi + 1) * 128],
                            tp[:],
                        )
                # U^T for this q-chunk
                for et in range(DT):
                    strip = p3s.tile([128, KVT, 128], F32R, tag="xs")
                    nc.sync.dma_start(
                        strip[:],
                        xkvS_d.ap()[et].rearrange("(kvt p) c -> p kvt c", p=128),
                    )
                    u_ps = ps_uo.tile([128, NCH], F32, tag="x")
                    for kvt in range(KVT):
                        nc.tensor.matmul(
                            u_ps[:],
                            strip[:, kvt, :],
                            PT_sb[:, kvt * NCH:(kvt + 1) * NCH],
                            start=(kvt == 0),
                            stop=(kvt == KVT - 1),
                        )
                    nc.vector.tensor_copy(
                        UT_sb[:, et * SQ + qc * NCH: et * SQ + (qc + 1) * NCH],
                        u_ps[:],
                    )

        # ============ phase 4: O = (U @ Wv) / rowsum ============
        with tc.tile_pool(name=f"p4{rep}", bufs=1) as p4, \
             tc.tile_pool(name=f"p4o{rep}", bufs=4) as p4o:
            wv_sb = p4.tile([128, DT * D], F32R, tag="wv")
            for et in range(DT):
                nc.sync.dma_start(
                    wv_sb[:, et * D:(et + 1) * D],
                    wv_d.ap()[et * 128:(et + 1) * 128, :],
                )
            for qt in range(QT):
                for dc in range(NDC):
                    o_ps = ps_uo.tile([128, NCH], F32, tag="x")
                    for et in range(DT):
                        nc.tensor.matmul(
                            o_ps[:],
                            UT_sb[:, et * SQ + qt * 128: et * SQ + (qt + 1) * 128],
                            wv_sb[:, et * D + dc * NCH: et * D + (dc + 1) * NCH],
                            start=(et == 0),
                            stop=(et == DT - 1),
                        )
                    o_sb = p4o.tile([128, NCH], F32, tag="o")
                    nc.scalar.mul(o_sb[:], o_ps[:], mul=recip_sb[:, qt:qt + 1])
                    nc.sync.dma_start(
                        out_d.ap()[qt * 128:(qt + 1) * 128, dc * NCH:(dc + 1) * NCH],
                        o_sb[:],
                    )


_NC_CACHE = None


def get_nc():
    global _NC_CACHE
    if _NC_CACHE is None:
        _NC_CACHE = build_nc()
    return _NC_CACHE


def make_in_maps(inputs, W_query, W_key, W_value):
    x = np.ascontiguousarray(np.asarray(inputs, dtype=np.float32))
    Wq = np.asarray(W_query, dtype=np.float32)
    Wk = np.asarray(W_key, dtype=np.float32)
    Wv = np.ascontiguousarray(np.asarray(W_value, dtype=np.float32))

    # weight folding on host: A = Wq @ Wk^T (fp64 accumulate, fp32 store)
    A = (Wq.astype(np.float64) @ Wk.astype(np.float64).T).astype(np.float32)

    in_maps = []
    for b in range(B):
        for h in range(2):
            # roll kv so this core's SQ query rows sit at kv[0:SQ]
            xb = x[b]
            if h == 1:
                xb = np.concatenate([xb[SQ:], xb[:SQ]], axis=0)
            xb = np.ascontiguousarray(xb)
            xkvT = np.ascontiguousarray(xb.T)              # [D, SKV]
            xkvS = np.ascontiguousarray(
                xb.reshape(SKV, DT, 128).transpose(1, 0, 2)
            )                                              # [DT, SKV, 128]
            in_maps.append({
                "A": A, "wv": Wv,
                "xkvT": xkvT, "xkvS": xkvS,
            })
    return in_maps


def kernel(inputs, W_query, W_key, W_value):
    nc = get_nc()
    in_maps = make_in_maps(inputs, W_query, W_key, W_value)
    res = run_bass_kernel_spmd(nc, in_maps, core_ids=list(range(8)))
    out = np.empty((B, S, D), dtype=np.float32)
    for b in range(B):
        for h in range(2):
            out[b, h * SQ:(h + 1) * SQ, :] = res.results[2 * b + h]["out"]
    return out
